# revision 13
# baseline (speedup 1.0000x reference)
"""Trainium2 Bass kernel for full-dim attention — float32r variant.

Folded algorithm (scores = x M x.T / 32 with M = wq.T wk;
out = (p x) W2.T with W2 = wo wv) but all matmuls run in float32r
(single-pass reduced-precision fp32, full rate at N>=256), giving ~3e-4
relative error instead of bf16's ~4e-3 (bf16 variant: kernel_bf16.py).

fp32 operands double the SBUF footprint, so unlike the bf16 kernel not
everything is resident:
  - xTq  [128, 8, 1024]  query half of x.T, resident (uT rhs + scores lhsT)
  - xTk  [8][128, 8, 128] key-half scores lhsT tiles, streamed per key tile
  - xn2  [8][128, 16, 128] natural-x px lhsT tiles, streamed per d tile
  - pT / pxT processed in two 512-query halves (pool slots reused)
Chunks consumed twice (both query halves) are re-DMAed rather than held.
"""

import sys

if "/opt/trn_rl_repo" not in sys.path:
    sys.path.insert(0, "/opt/trn_rl_repo")

import numpy as np
import ml_dtypes

N_CORES = 8
P = 128

_BUILD_CACHE = {}


def _build(S, D, SQ):
    import concourse.mybir as mybir
    import concourse.tile as tile
    from concourse import bacc

    key = (S, D, SQ)
    if key in _BUILD_CACHE:
        return _BUILD_CACHE[key]

    dt = mybir.dt
    DS = D // P           # d subtiles
    SKT = S // P          # key tiles
    SQT = SQ // P         # query tiles
    HSKT = SKT - SQT      # key tiles beyond the query half
    NB = 512
    NBg = min(NB, D)
    GB = D // NBg
    NH = max(1, SQ // NB)          # query halves processed separately
    HW_ = SQ // NH                 # queries per half
    HT = HW_ // P                  # query tiles per half
    INV_SQRT_D = 1.0 / float(np.sqrt(np.float32(D)))

    nc = bacc.Bacc(None, target_bir_lowering=False, debug=False)

    f32r = dt.float32r
    xTq_d = nc.dram_tensor("xTq", [P, DS, SQ], f32r, kind="ExternalInput")
    xTk_d = nc.dram_tensor("xTk", [HSKT, P, DS, P], f32r, kind="ExternalInput")
    xn2_d = nc.dram_tensor("xn2", [DS, P, SKT, P], f32r, kind="ExternalInput")
    mT_d = nc.dram_tensor("mT", [P, DS, D], f32r, kind="ExternalInput")
    w2T_d = nc.dram_tensor("w2T", [P, DS, D], f32r, kind="ExternalInput")
    y_d = nc.dram_tensor("y", [SQ, D], dt.float32, kind="ExternalOutput")

    with tile.TileContext(nc) as tc:
        with (
            tc.tile_pool(name="xTq", bufs=1) as xTq_pool,
            tc.tile_pool(name="xTk", bufs=4) as xTk_pool,
            tc.tile_pool(name="xn2", bufs=3) as xn2_pool,
            tc.tile_pool(name="w", bufs=1) as w_pool,        # mT then w2T
            tc.tile_pool(name="uT", bufs=1) as uT_pool,
            tc.tile_pool(name="pTh", bufs=1) as pT_pool,
            tc.tile_pool(name="pxh", bufs=1) as px_pool,
            tc.tile_pool(name="stat", bufs=1) as stat_pool,
            tc.tile_pool(name="outsb", bufs=2) as out_pool,
            tc.tile_pool(name="ps", bufs=6, space="PSUM") as ps_pool,
            tc.tile_pool(name="zps", bufs=2, space="PSUM") as z_pool,
        ):
            xTq = xTq_pool.tile([P, DS, SQ], f32r)
            mT = w_pool.tile([P, DS, D], f32r, tag="w", name="mT")

            S1 = stat_pool.tile([P, SQ], dt.float32)
            # ones (col 0) and 1/Z per query tile (cols 8..) share one tile
            zs = stat_pool.tile([P, 8 + SQT], dt.float32, name="zs")
            nc.vector.memset(zs[:, 0:1], 1.0)

            # PE warmup (bf16, independent of the fp32r data path)
            wrm = stat_pool.tile([P, P], dt.bfloat16, name="wrm")
            nc.vector.memset(wrm[:], 0.0)
            wps = z_pool.tile([P, P], dt.float32, tag="zp", name="wps")
            for i in range(36):
                nc.tensor.matmul(wps[:], wrm[:], wrm[:], start=True, stop=True)

            # ship exactly the first uT sweep's operands first (dt_<4
            # halves of mT, first 512 query columns of xTq), then the
            # second-sweep pieces in consumption order
            MH = min(4, DS) * P
            XH = min(NB, SQ)
            for ds in range(DS):
                nc.sync.dma_start(mT[:, ds, :MH], mT_d[:, ds, :MH])
                nc.sync.dma_start(xTq[:, ds, :XH], xTq_d[:, ds, :XH])
            for ds in range(DS):
                if XH < SQ:
                    nc.sync.dma_start(xTq[:, ds, XH:], xTq_d[:, ds, XH:])
            for ds in range(DS):
                nc.sync.dma_start(mT[:, ds, MH:], mT_d[:, ds, MH:])

            # ---- uT[d', sq] = sum_d mT[d, d'] xTq[d, sq] ----
            # ds-major sweeps over groups of 4 d'-tiles: the first sweep
            # consumes the mT/xTq chunks in DMA arrival order, so the PE
            # ramps with the input stream instead of stalling on the full
            # 8 MB before its first iteration completes.
            uT = uT_pool.tile([P, DS, SQ], f32r)
            NBq = min(NB, SQ)
            DTG = min(4, DS)
            for g in range(DS // DTG):
                for sqb in range(SQ // NBq):
                    grp = range(g * DTG, (g + 1) * DTG)
                    pss = [ps_pool.tile([P, NBq], dt.float32, tag="ps", name=f"ps_u{sqb}_{g}_{j}") for j in range(DTG)]
                    for ds in range(DS):
                        for j, dt_ in enumerate(grp):
                            nc.tensor.matmul(
                                pss[j][:], mT[:, ds, dt_ * P:(dt_ + 1) * P],
                                xTq[:, ds, sqb * NBq:(sqb + 1) * NBq],
                                start=(ds == 0), stop=(ds == DS - 1),
                            )
                    for j, dt_ in enumerate(grp):
                        nc.any.tensor_copy(uT[:, dt_, sqb * NBq:(sqb + 1) * NBq], pss[j][:])

            # w2T loads into the slot mT frees after the uT phase; its
            # chunk DMAs are interleaved into the DMA-free first skt
            # iterations of the h=0 scores loop below so they don't queue
            # ahead of the streamed xTk chunks.
            w2T = w_pool.tile([P, DS, D], f32r, tag="w", name="w2T")

            for h in range(NH):
                hq = h * HW_                       # query offset of this half

                # ---- pT[sk, sq_h] = exp(x u.T / 32) ----
                pTh = pT_pool.tile([P, SKT, HW_], f32r, tag="pT", name=f"pT{h}")
                for skt in range(SKT):
                    if h == 0 and skt < DS:
                        nc.sync.dma_start(w2T[:, skt, :], w2T_d[:, skt, :])
                    if skt < SQT:
                        xkc = None
                    else:
                        xkc = xTk_pool.tile([P, DS, P], f32r, tag="xtk",
                                            name=f"xtk{h}_{skt}")
                        nc.sync.dma_start(xkc[:], xTk_d[skt - SQT])
                    ps1 = ps_pool.tile([P, HW_], dt.float32, tag="ps",
                                       name=f"ps_s{h}_{skt}")
                    for ds in range(DS):
                        if xkc is None:
                            lhs = xTq[:, ds, skt * P:(skt + 1) * P]
                        else:
                            lhs = xkc[:, ds, :]
                        nc.tensor.matmul(
                            ps1[:], lhs, uT[:, ds, hq:hq + HW_],
                            start=(ds == 0), stop=(ds == DS - 1),
                        )
                    nc.scalar.activation(
                        pTh[:, skt, :], ps1[:],
                        mybir.ActivationFunctionType.Exp, scale=INV_SQRT_D,
                    )
                    if skt == 0:
                        nc.vector.tensor_copy(S1[:, hq:hq + HW_], pTh[:, 0, :])
                    else:
                        nc.vector.tensor_add(S1[:, hq:hq + HW_],
                                             S1[:, hq:hq + HW_], pTh[:, skt, :])

                # ---- Z and 1/Z for this half ----
                for t in range(HT):
                    sqt = h * HT + t
                    zp = z_pool.tile([P, 1], dt.float32, tag="zp", name=f"zp{sqt}")
                    nc.tensor.matmul(zp[:], S1[:, sqt * P:(sqt + 1) * P],
                                     zs[:, 0:1], start=True, stop=True)
                    nc.vector.reciprocal(zs[:, 8 + sqt:9 + sqt], zp[:])

                # ---- pxT[d, sq_h] = sum_sk xN[sk, d] pT[sk, sq_h] ----
                pxh = px_pool.tile([P, DS, HW_], f32r, tag="px", name=f"px{h}")
                for dt_ in range(DS):
                    xnc = xn2_pool.tile([P, SKT, P], f32r, tag="xn",
                                        name=f"xn{h}_{dt_}")
                    nc.sync.dma_start(xnc[:, :SKT // 2, :], xn2_d[dt_, :, :SKT // 2, :])
                    nc.sync.dma_start(xnc[:, SKT // 2:, :], xn2_d[dt_, :, SKT // 2:, :])
                    ps2 = ps_pool.tile([P, HW_], dt.float32, tag="ps",
                                       name=f"ps_c{h}_{dt_}")
                    for skt in range(SKT):
                        nc.tensor.matmul(
                            ps2[:], xnc[:, skt, :], pTh[:, skt, :],
                            start=(skt == 0), stop=(skt == SKT - 1),
                        )
                    nc.any.tensor_copy(pxh[:, dt_, :], ps2[:])

                # ---- y[sq_h, g] = (sum_d pxT[d, sq_h] w2T[d, g]) / Z ----
                for t in range(HT):
                    sqt = h * HT + t
                    pss = [ps_pool.tile([P, NBg], dt.float32, tag="ps", name=f"ps_o{sqt}_{i}") for i in range(GB)]
                    for ds in range(DS):
                        lhs = pxh[:, ds, t * P:(t + 1) * P]
                        for gb in range(GB):
                            nc.tensor.matmul(
                                pss[gb][:], lhs, w2T[:, ds, gb * NBg:(gb + 1) * NBg],
                                start=(ds == 0), stop=(ds == DS - 1),
                            )
                    for gb in range(GB):
                        ot = out_pool.tile([P, NBg], dt.float32, tag="ot",
                                           name=f"ot{sqt}_{gb}")
                        nc.vector.tensor_mul(
                            ot[:], pss[gb][:],
                            zs[:, 8 + sqt:9 + sqt].to_broadcast([P, NBg]))
                        nc.sync.dma_start(
                            y_d[sqt * P:(sqt + 1) * P, gb * NBg:(gb + 1) * NBg],
                            ot[:])

    nc.compile()
    _BUILD_CACHE[key] = nc
    return nc


def _prep_T32(a_T, n_sub):
    """[k, n] fp32 -> [128, k/128, n] fp32 (k on partitions)."""
    k, n = a_T.shape
    return np.ascontiguousarray(
        a_T.reshape(n_sub, P, n).transpose(1, 0, 2).astype(np.float32))


def _run(x, wq, wk, wv, wo, trace=False):
    from concourse.bass_utils import run_bass_kernel_spmd

    B, S, D = x.shape
    SQ = B * S // N_CORES
    halves = S // SQ
    DS = D // P
    SKT = S // P
    nc = _build(S, D, SQ)

    x = np.asarray(x, dtype=np.float32)
    wq = np.asarray(wq, dtype=np.float32)
    wk = np.asarray(wk, dtype=np.float32)
    wv = np.asarray(wv, dtype=np.float32)
    wo = np.asarray(wo, dtype=np.float32)
    M = wq.T @ wk
    W2 = wo @ wv
    mT = _prep_T32(M, DS)
    w2T = _prep_T32(np.ascontiguousarray(W2.T), DS)

    in_maps = []
    for c in range(N_CORES):
        b, h = divmod(c, halves)
        xb = x[b]
        if h != 0:
            xb = np.concatenate([xb[h * SQ:(h + 1) * SQ], xb[:h * SQ],
                                 xb[(h + 1) * SQ:]], axis=0)
        xb = np.ascontiguousarray(xb, dtype=np.float32)
        # query half of x.T, d on partitions: [128, DS, SQ]
        xTq = np.ascontiguousarray(
            xb[:SQ].T.reshape(DS, P, SQ).transpose(1, 0, 2))
        # key-half scores lhsT tiles: [HSKT, 128, DS, 128]
        xk = xb[SQ:]
        hskt = xk.shape[0] // P
        xTk = np.ascontiguousarray(
            xk.T.reshape(DS, P, hskt, P).transpose(2, 1, 0, 3))
        # natural-x px lhsT tiles, d-major: [DS, 128, SKT, 128]
        xn2 = np.ascontiguousarray(
            xb.reshape(SKT, P, DS, P).transpose(2, 1, 0, 3))
        in_maps.append({"xTq": xTq, "xTk": xTk, "xn2": xn2,
                        "mT": mT, "w2T": w2T})

    res = run_bass_kernel_spmd(nc, in_maps, core_ids=list(range(N_CORES)),
                               trace=trace)
    out = np.empty((B, S, D), dtype=np.float32)
    for c in range(N_CORES):
        b, h = divmod(c, halves)
        out[b, h * SQ:(h + 1) * SQ, :] = res.results[c]["y"]
    return out, res


def kernel(x, wq, wk, wv, wo):
    out, _ = _run(x, wq, wk, wv, wo)
    return out
